# revision 2
# baseline (speedup 1.0000x reference)
# Multi-head causal attention (B=4, T=2048, D=1024, H=16) on 8 TRN2 NeuronCores.
#
# Sharding: data-parallel over the 4 batches x 2-way split of query rows
# (interleaved 128-row blocks for causal load balance). K/V projections are
# SPLIT between the two cores of a batch pair: each core projects K/V only
# for its own 1024 tokens, then the pair exchanges K/V via two chunked
# AllGathers (2 MB each) overlapped with Q projection and early attention.
#
# SPMD trick: K/V live in SBUF in RANK-MAJOR block order (k_sb[..., r, :]
# = rank r's blocks; rank 0 owns even global 128-token blocks, rank 1 odd),
# so all addressing is identical on every core; global block (2kk+i) is
# (rank i, block kk). Attention sums over tk blocks are order-independent,
# and masks are host-fed per-core.
#
# Per-core kernel (all matmul operands bf16, fp32 PSUM accumulation):
#   qT/kT = W @ x^T (per head-pair group), V kept (t, d)-major with an extra
#   ones column so the attention-value matmul also produces the softmax
#   denominator. Scores are computed transposed (tk partition, tq free),
#   exp on the Scalar engine (no max subtraction: |scores| <= ~3), causal
#   masking via multiplicative {0,1} mask tiles fed per-core, denominator
#   broadcast across partitions on GpSimd.
import numpy as np
import ml_dtypes

B, T, D, H, DH, P = 4, 2048, 1024, 16, 64, 128
NQ = 1024          # query tokens per core
NCORES = 8
BF16 = ml_dtypes.bfloat16

_COMPILED = {}


def _build_nc():
    from contextlib import ExitStack
    from functools import partial
    import concourse.mybir as mybir
    import concourse.tile as tile
    from concourse import bacc

    bf = mybir.dt.bfloat16
    f32 = mybir.dt.float32
    EXP = mybir.ActivationFunctionType.Exp

    nc = bacc.Bacc("TRN2", target_bir_lowering=False, debug=False,
                   num_devices=NCORES)

    # ---- DRAM I/O ----
    xq_d = nc.dram_tensor("xq", [D, NQ], bf, kind="ExternalInput").ap()
    wq_d = nc.dram_tensor("wqT", [D, D], bf, kind="ExternalInput").ap()
    wk_d = nc.dram_tensor("wkT", [D, D], bf, kind="ExternalInput").ap()
    wv_d = nc.dram_tensor("wvT", [D, D], bf, kind="ExternalInput").ap()
    wo_d = nc.dram_tensor("woT", [D, D], bf, kind="ExternalInput").ap()
    bq_d = nc.dram_tensor("bq_r", [P, 8], f32, kind="ExternalInput").ap()
    bk_d = nc.dram_tensor("bk_r", [P, 8], f32, kind="ExternalInput").ap()
    bo_d = nc.dram_tensor("bo_r", [P, 8], f32, kind="ExternalInput").ap()
    bv_d = nc.dram_tensor("bv_bc", [P, D], bf, kind="ExternalInput").ap()
    cm_d = nc.dram_tensor("cmask", [2, P, 1024], bf, kind="ExternalInput").ap()
    y_d = nc.dram_tensor("yT", [D, NQ], f32, kind="ExternalOutput").ap()

    xq_r = xq_d.rearrange("(g p) t -> p g t", p=P)

    with tile.TileContext(nc) as tc, ExitStack() as ctx:
        const = ctx.enter_context(tc.tile_pool(name="const", bufs=1))
        xchunk = ctx.enter_context(tc.tile_pool(name="xchunk", bufs=1))
        kvst = ctx.enter_context(tc.tile_pool(name="kvst", bufs=1))
        expps = ctx.enter_context(tc.tile_pool(name="expps", bufs=4))
        small = ctx.enter_context(tc.tile_pool(name="small", bufs=2))
        ps_s = ctx.enter_context(tc.tile_pool(name="ps_s", bufs=2, space="PSUM"))
        ps_av = ctx.enter_context(tc.tile_pool(name="ps_av", bufs=1, space="PSUM"))
        ps_m = ctx.enter_context(tc.tile_pool(name="ps_m", bufs=2, space="PSUM"))
        dram = ctx.enter_context(tc.tile_pool(name="dram", bufs=2, space="DRAM"))

        # ---- resident SBUF tensors ----
        # wq shares the wk buffer (loaded after K proj); wo shares wv's.
        wk_sb = const.tile([P, 8, 8, P], bf, name="wqk", tag="wqk")
        wv_sb = const.tile([P, 8, D], bf, name="wvo", tag="wvo")
        k_sb = const.tile([P, 8, 2, 1024], bf)   # [k, g, rank, blk*128]
        q_sb = const.tile([P, 8, NQ], bf)
        v1_sb = const.tile([P, 2, 8, 16, 65], bf)  # [tk, rank, blk, head, V|1]
        a_sb = const.tile([P, 8, NQ], bf)        # attention out (d, tq)
        mk_sb = const.tile([P, 2, 1024], bf)
        bq_sb = const.tile([P, 8], f32)
        bk_sb = const.tile([P, 8], f32)
        bo_sb = const.tile([P, 8], f32)
        bv_sb = const.tile([P, D], bf)

        dma = nc.sync.dma_start
        mm = nc.tensor.matmul

        def _v_group(xc, vl, dhalf, ti):
            ps = ps_m.tile([P, 512], f32, name="ps", tag="ps")
            for kg in range(8):
                mm(ps, lhsT=xc[:, kg, ti * P:(ti + 1) * P],
                   rhs=wv_sb[:, kg, dhalf * 512:(dhalf + 1) * 512],
                   start=(kg == 0), stop=(kg == 7))
            nc.vector.tensor_add(
                out=vl[:, ti, 8 * dhalf:8 * dhalf + 8, 0:64],
                in0=ps.rearrange("p (h c) -> p h c", c=64),
                in1=bv_sb[:, dhalf * 512:(dhalf + 1) * 512]
                    .rearrange("p (h c) -> p h c", c=64))

        def _k_group(xc, kl, g):
            ps = ps_m.tile([P, 512], f32, name="ps", tag="ps")
            for kg in range(8):
                mm(ps, lhsT=wk_sb[:, kg, g, :], rhs=xc[:, kg, :],
                   start=(kg == 0), stop=(kg == 7))
            nc.vector.tensor_scalar_add(out=kl[:, g, :], in0=ps,
                                        scalar1=bk_sb[:, g:g + 1])

        def _q_group(xc, wq_sb, ts, g):
            ps = ps_m.tile([P, 512], f32, name="ps", tag="ps")
            for kg in range(8):
                mm(ps, lhsT=wq_sb[:, kg, g, :], rhs=xc[:, kg, :],
                   start=(kg == 0), stop=(kg == 7))
            nc.vector.tensor_scalar_add(
                out=q_sb[:, g, ts * 512:(ts + 1) * 512], in0=ps,
                scalar1=bq_sb[:, g:g + 1])

        def kv_chunk(ts):
            # project K/V for OWN tokens [ts*512, (ts+1)*512) into staging
            xc = xchunk.tile([P, 8, 512], bf, name=f"xc{ts}", tag=f"xc{ts}",
                             bufs=1)
            dma(out=xc, in_=xq_r[:, :, ts * 512:(ts + 1) * 512])
            vl = kvst.tile([P, 4, 16, 65], bf, name="vl", tag="vl", bufs=1)
            kl = kvst.tile([P, 8, 512], bf, name="kl", tag="kl", bufs=1)
            nc.vector.memset(vl[:, :, :, 64:65], 1.0)
            th = [partial(_v_group, xc, vl, dh, ti)
                  for dh in range(2) for ti in range(4)]
            th += [partial(_k_group, xc, kl, g) for g in range(8)]
            return xc, vl, kl, th

        def kv_comm(kl, vl):
            # bounce own K/V chunk to DRAM, AllGather with pair core
            kvi = dram.tile([P, 8256], bf, name="kvi", tag="kvi", bufs=2)
            dma(out=kvi[:, 0:4096].rearrange("p (g c) -> p g c", c=512),
                in_=kl)
            dma(out=kvi[:, 4096:8256]
                .rearrange("p (b h c) -> p b h c", h=16, c=65), in_=vl)
            kvo = dram.tile([2, P, 8256], bf, name="kvo", tag="kvo", bufs=2)
            nc.gpsimd.collective_compute(
                "AllGather", mybir.AluOpType.bypass,
                replica_groups=[[0, 1], [2, 3], [4, 5], [6, 7]],
                ins=[kvi.opt()], outs=[kvo.opt()])
            return kvo

        def kv_in(ts, kvo):
            # unpack both ranks' chunks into rank-major K/V SBUF layout
            for r in (0, 1):
                dma(out=k_sb[:, :, r, ts * 512:(ts + 1) * 512],
                    in_=kvo[r, :, 0:4096].rearrange("p (g c) -> p g c", c=512))
                dma(out=v1_sb[:, r, 4 * ts:4 * ts + 4, :, :],
                    in_=kvo[r, :, 4096:8256]
                    .rearrange("p (b h c) -> p b h c", h=16, c=65))

        def attn_slot(g, j):
            # one accumulator bank PER HEAD: PSUM accumulation groups must be
            # bank-aligned on HW (a group at a 256-col offset inside a bank
            # silently corrupts). Row 64 collects the softmax denominator
            # via the ones column of v1_sb.
            pav = [ps_av.tile([65, 256], f32, tag=f"pav{c}",
                              name=f"pav{c}") for c in (0, 1)]
            for kk in range(2 * j + 2):  # within-rank tk block index
                ps = ps_s.tile([P, 1024], f32, name="scps", tag="scps")
                expp = expps.tile([P, 1024], bf, name="expp", tag="expp")
                for c in (0, 1):         # head within pair
                    for i in (0, 1):     # rank (global block 2kk+i)
                        mm(ps[:, c * 512 + i * 256: c * 512 + i * 256 + 256],
                           lhsT=k_sb[64 * c:64 * c + 64, g, i,
                                     kk * P:(kk + 1) * P],
                           rhs=q_sb[64 * c:64 * c + 64, g,
                                    j * 256:(j + 1) * 256],
                           start=True, stop=True,
                           tile_position=(64 * c, 0))
                nc.scalar.activation(out=expp, in_=ps, func=EXP, scale=0.125)
                if kk >= 2 * j:
                    nc.vector.tensor_mul(expp, expp, mk_sb[:, kk - 2 * j, :])
                for c in (0, 1):
                    for i in (0, 1):
                        mm(pav[c],
                           lhsT=v1_sb[:, i, kk, 2 * g + c, :],
                           rhs=expp[:, c * 512 + i * 256:
                                    c * 512 + i * 256 + 256],
                           start=(kk == 0 and i == 0),
                           stop=(kk == 2 * j + 1 and i == 1))
            # copy accumulators to SBUF right away so the PSUM banks free up
            # for the next slot; normalize runs off the PE critical path
            av = [small.tile([65, 256], f32, tag=f"av{c}", bufs=2,
                             name=f"av{c}") for c in (0, 1)]
            for c in (0, 1):
                nc.vector.tensor_copy(out=av[c], in_=pav[c])
            # both heads' denominators into one partition-base-0 tile
            # (reciprocal_approx_fast corrupts base!=0 inputs on HW)
            den2 = small.tile([1, 512], f32, tag="den2", bufs=1, name="den2")
            for c in (0, 1):
                nc.vector.tensor_copy(out=den2[:, c * 256:(c + 1) * 256],
                                      in_=av[c][64:65, :])
            rec = small.tile([1, 512], f32, tag="rec", bufs=1, name="rec")
            nc.vector.reciprocal_approx_fast(out=rec, in_=den2)
            sbb = small.tile([64, 512], f32, tag="sbb", name="sbb")
            nc.gpsimd.partition_broadcast(sbb, rec)
            for c in (0, 1):
                nc.vector.tensor_mul(
                    out=a_sb[64 * c:64 * c + 64, g, j * 256:(j + 1) * 256],
                    in0=av[c][0:64, :], in1=sbb[:, c * 256:(c + 1) * 256])

        def wo_group(wo_sb, q4, o):
            # 256-wide output-projection chunk: needs only attention slot q4
            ps = ps_m.tile([P, 512], f32, name="ps", tag="ps")
            for g in range(8):
                mm(ps[:, 0:256], lhsT=wo_sb[:, g, o * P:(o + 1) * P],
                   rhs=a_sb[:, g, q4 * 256:(q4 + 1) * 256],
                   start=(g == 0), stop=(g == 7))
            ysb = small.tile([P, 512], f32, tag="ysb", name="ysb")
            nc.vector.tensor_scalar_add(out=ysb[:, 0:256], in0=ps[:, 0:256],
                                        scalar1=bo_sb[:, o:o + 1])
            dma(out=y_d[o * P:(o + 1) * P, q4 * 256:(q4 + 1) * 256],
                in_=ysb[:, 0:256])

        def drain(pend, n):
            for _ in range(min(n, len(pend))):
                pend.pop(0)()

        # Emission order sets PE priority: the attention inner loop is paced
        # by the ScalarE exp chain; feed PE projection/output-proj groups
        # BETWEEN attention slots to keep it busy. Fire the two AllGathers
        # as early as possible so chunk-0 K/V lands before attention starts.
        dma(out=bv_sb, in_=bv_d)
        dma(out=wv_sb, in_=wv_d.rearrange("(kg p) d -> p kg d", p=P))
        dma(out=wk_sb, in_=wk_d.rearrange("(kg p) (g c) -> p kg g c", p=P, c=P))
        dma(out=bk_sb, in_=bk_d)
        dma(out=bq_sb, in_=bq_d)
        xc0, vl0, kl0, th0 = kv_chunk(0)
        for t in th0:
            t()                            # V0 + K0 projections
        kvo0 = kv_comm(kl0, vl0)           # AG1
        xc1, vl1, kl1, th1 = kv_chunk(1)
        for t in th1:
            t()                            # V1 + K1
        kvo1 = kv_comm(kl1, vl1)           # AG2
        wq_sb = const.tile([P, 8, 8, P], bf, name="wqk", tag="wqk")
        dma(out=wq_sb, in_=wq_d.rearrange("(kg p) (g c) -> p kg g c", p=P, c=P))
        kv_in(0, kvo0)
        kv_in(1, kvo1)
        dma(out=mk_sb, in_=cm_d.rearrange("m p c -> p m c"))
        dma(out=bo_sb, in_=bo_d)
        wo_sb = const.tile([P, 8, D], bf, name="wvo", tag="wvo")
        dma(out=wo_sb, in_=wo_d.rearrange("(g p) d -> p g d", p=P))

        pq0 = [partial(_q_group, xc0, wq_sb, 0, g) for g in range(8)]
        drain(pq0, 99)                     # Q chunk 0
        # attn slot j needs K/V global blocks 0..4j+3 and Q chunk j//2.
        # Wo chunk q4 needs attention slot q4 of all groups.
        pend = [partial(_q_group, xc1, wq_sb, 1, g) for g in range(8)]
        for g in range(8):
            attn_slot(g, 0)
            drain(pend, 1)
        drain(pend, 99)
        pend = [partial(wo_group, wo_sb, 0, o) for o in range(8)]
        for g in range(8):
            attn_slot(g, 1)
            drain(pend, 1)
        drain(pend, 99)
        # j=3 before j=2 so Wo chunks 1 and 3 both get an exp stretch to hide in
        pend = [partial(wo_group, wo_sb, 1, o) for o in range(8)]
        for g in range(8):
            attn_slot(g, 3)
            drain(pend, 1)
        drain(pend, 99)
        pend = [partial(wo_group, wo_sb, 3, o) for o in range(8)]
        for g in range(8):
            attn_slot(g, 2)
            drain(pend, 1)
        drain(pend, 99)
        for o in range(8):
            wo_group(wo_sb, 2, o)

    nc.compile()
    return nc


def _get_nc():
    if "nc" not in _COMPILED:
        _COMPILED["nc"] = _build_nc()
    return _COMPILED["nc"]


def _core_token_blocks(par):
    return [2 * i + par for i in range(8)]


def _masks(par):
    tri = np.triu(np.ones((P, P), np.float32))   # keep tk <= tq
    on = np.ones((P, P), np.float32)
    z = np.zeros((P, P), np.float32)
    if par == 0:
        m = [np.concatenate(r, axis=1)
             for r in [[tri, on], [z, on], [z, tri], [z, z]]]
    else:
        m = [np.concatenate(r, axis=1)
             for r in [[on, on], [tri, on], [z, on], [z, tri]]]
    # combined group masks matching expp layout [k h0 | k+1 h0 | k h1 | k+1 h1]
    row0 = np.concatenate([m[0], m[1], m[0], m[1]], axis=1)
    row1 = np.concatenate([m[2], m[3], m[2], m[3]], axis=1)
    return np.stack([row0, row1]).astype(BF16)


def _make_in_maps(x, wq, bq, wk, bk, wv, bv, wo, bo):
    bfc = lambda a: np.ascontiguousarray(np.asarray(a, np.float32).T).astype(BF16)
    shared = {
        "wqT": bfc(wq), "wkT": bfc(wk), "wvT": bfc(wv), "woT": bfc(wo),
        "bq_r": np.ascontiguousarray(np.asarray(bq, np.float32).reshape(8, P).T),
        "bk_r": np.ascontiguousarray(np.asarray(bk, np.float32).reshape(8, P).T),
        "bo_r": np.ascontiguousarray(np.asarray(bo, np.float32).reshape(8, P).T),
        "bv_bc": np.ascontiguousarray(
            np.broadcast_to(np.asarray(bv, np.float32).astype(BF16), (P, D))),
    }
    masks = [_masks(0), _masks(1)]
    in_maps, idx_list = [], []
    for core in range(NCORES):
        b, par = core // 2, core % 2
        blocks = _core_token_blocks(par)
        idx = np.concatenate([np.arange(P * blk, P * blk + P) for blk in blocks])
        xT = np.asarray(x[b], np.float32).T
        m = dict(shared)
        m["xq"] = np.ascontiguousarray(xT[:, idx]).astype(BF16)
        m["cmask"] = masks[par]
        in_maps.append(m)
        idx_list.append((b, idx))
    return in_maps, idx_list


def _run(inputs, trace=False):
    from concourse.bass_utils import run_bass_kernel_spmd
    nc = _get_nc()
    in_maps, idx_list = _make_in_maps(**inputs)
    res = run_bass_kernel_spmd(nc, in_maps, list(range(NCORES)), trace=trace)
    y = np.empty((B, T, D), np.float32)
    for core in range(NCORES):
        b, idx = idx_list[core]
        y[b][idx, :] = res.results[core]["yT"].T
    return y, res


def kernel(**inputs):
    y, _ = _run(inputs, trace=False)
    return y
